# revision 1
# baseline (speedup 1.0000x reference)
"""MoE expert-MLP (SwiGLU) kernel for 8 Trainium2 NeuronCores.

Strategy: expert-parallel. Each of the 8 cores owns one expert's weights.
Tokens are routed on the host: every (token, k) routing slot is dispatched to
its expert's core, padded to a fixed per-expert capacity. Each core runs a
dense [cap, D] SwiGLU MLP for its expert in fp32r (full-rate fp32 matmul
mode on the PE array) and scales rows by the routing weight. The host then
scatter-combines the K=2 per-token contributions. No collectives needed.

Per-core kernel (cap tokens, D=2048, H=1408), loops in passes of <=768
tokens so x^T, h^T stay SBUF-resident per pass while Wg/Wu stream per
h-tile. Wd stays fully resident. The first pass is smaller so the PE can
start as soon as the first x^T d-chunk lands.
  stage A: h^T[h, t] = silu(Wg @ x^T) * (Wu @ x^T)   (PSUM accumulate over D)
  stage B: y[t, d]  = (h^T)^T @ Wd^T, row-scaled by routing weight
"""

import sys
import os

sys.path.insert(0, "/opt/trn_rl_repo")

import numpy as np

T, D, H, E, K = 8192, 2048, 1408, 8, 2
P = 128
HT = H // P        # 11 h-tiles
KT = D // P        # 16 d-tiles
DC = 512           # moving-dim chunk for stage B

_built = {}


def _pass_sizes(cap):
    """Split cap into passes: first ~640 (fast startup), rest 768.
    Every size is a multiple of 128 in [256, 768]."""
    sizes = []
    rem = cap
    while rem:
        if rem <= 768:
            s = rem
        elif rem - 768 >= 256:
            s = 768
        else:
            s = rem - 256
        sizes.append(s)
        rem -= s
    assert all(s % 128 == 0 and 256 <= s <= 768 for s in sizes), sizes
    return sizes


def _tg_split(s):
    """Split a pass into <=2 moving-dim groups, each in [256, 512]."""
    if s <= 512:
        return [s]
    return [s - 384, 384]


def _build_nc(cap):
    import concourse.bass as bass  # noqa: F401
    from concourse import bacc
    import concourse.mybir as mybir
    import concourse.tile as tile

    F32 = mybir.dt.float32
    F32R = mybir.dt.float32r
    Silu = mybir.ActivationFunctionType.Silu
    Mult = mybir.AluOpType.mult

    sizes = _pass_sizes(cap)

    nc = bacc.Bacc("TRN2", target_bir_lowering=False, debug=False)
    xT = nc.declare_dram_parameter("xT", [D, cap], F32R, isOutput=False)
    wg = nc.declare_dram_parameter("wg", [HT, P, KT * P], F32R, isOutput=False)
    wu = nc.declare_dram_parameter("wu", [HT, P, KT * P], F32R, isOutput=False)
    wd = nc.declare_dram_parameter("wd", [H, D], F32R, isOutput=False)
    wt = nc.declare_dram_parameter("wt", [cap], F32, isOutput=False)
    out = nc.declare_dram_parameter("out", [cap, D], F32, isOutput=True)

    with tile.TileContext(nc) as tc:
        with (
            tc.tile_pool(name="sbuf", bufs=1) as pool,
            tc.tile_pool(name="psum", bufs=1, space="PSUM") as pp,
        ):
            wd_ts = [None] * HT
            wt_t = None
            t0 = 0
            for pi, TC in enumerate(sizes):
                tgs = _tg_split(TC)
                # x^T for this pass, one tile per d-tile so the first
                # matmuls only wait on the first chunk's DMA
                xt_ts = []
                for dti in range(KT):
                    xt_1 = pool.tile([P, TC], F32R, tag=f"xt{dti}", bufs=1,
                                     name=f"xt{dti}")
                    nc.gpsimd.dma_start(
                        xt_1[:], xT[dti * P : (dti + 1) * P, t0 : t0 + TC]
                    )
                    xt_ts.append(xt_1)
                # h^T for this pass
                h_t = pool.tile([P, HT, TC], F32R, tag="ht", bufs=1)

                # ---- stage A: h^T = silu(g^T) * u^T ----
                for ht in range(HT):
                    wg_t = pool.tile([P, KT * P], F32R, tag="wgu", bufs=3)
                    nc.sync.dma_start(wg_t[:], wg[ht, :, :])
                    wu_t = pool.tile([P, KT * P], F32R, tag="wgu", bufs=3)
                    nc.sync.dma_start(wu_t[:], wu[ht, :, :])
                    if pi == 0:
                        # stream Wd piecewise during pass 0 so it doesn't
                        # contend with the Wg/Wu stream on one DGE FIFO
                        wdc = pool.tile([P, D], F32R, tag=f"wd{ht}", bufs=1,
                                        name=f"wdc{ht}")
                        nc.gpsimd.dma_start(
                            wdc[:], wd[ht * P : (ht + 1) * P, :]
                        )
                        wd_ts[ht] = wdc

                    psg = [pp.tile([P, g], F32, tag=f"g{i}", bufs=1,
                                   name=f"psg{i}") for i, g in enumerate(tgs)]
                    psu = [pp.tile([P, g], F32, tag=f"u{i}", bufs=1,
                                   name=f"psu{i}") for i, g in enumerate(tgs)]
                    off = [0, tgs[0]]
                    for d in range(KT):
                        lhs = wg_t[:, d * P : (d + 1) * P]
                        for tg, g in enumerate(tgs):
                            nc.tensor.matmul(
                                psg[tg][:],
                                lhs,
                                xt_ts[d][:, off[tg] : off[tg] + g],
                                start=(d == 0),
                                stop=(d == KT - 1),
                            )
                    silu_ts = []
                    for tg, g in enumerate(tgs):
                        st = pool.tile([P, g], F32, tag="silu", bufs=2,
                                       name="st")
                        nc.scalar.activation(st[:], psg[tg][:], Silu)
                        silu_ts.append(st)
                    for d in range(KT):
                        lhs = wu_t[:, d * P : (d + 1) * P]
                        for tg, g in enumerate(tgs):
                            nc.tensor.matmul(
                                psu[tg][:],
                                lhs,
                                xt_ts[d][:, off[tg] : off[tg] + g],
                                start=(d == 0),
                                stop=(d == KT - 1),
                            )
                    for tg, g in enumerate(tgs):
                        nc.vector.tensor_tensor(
                            h_t[:, ht, off[tg] : off[tg] + g],
                            silu_ts[tg][:],
                            psu[tg][:],
                            op=Mult,
                        )

                if pi == 0:
                    wt_t = pool.tile([P, cap // P], F32, tag="wt", bufs=1)
                    nc.gpsimd.dma_start(
                        wt_t[:], wt.rearrange("(n p) -> p n", p=P)
                    )

                # ---- stage B: y = h @ Wd^T, scaled by routing weight ----
                for ts_ in range(TC // P):
                    psy = [pp.tile([P, DC], F32, tag=f"y{i}", bufs=1,
                                   name=f"psy{i}") for i in range(4)]
                    for ht in range(HT):
                        lhs = h_t[:, ht, ts_ * P : (ts_ + 1) * P]
                        for dc in range(4):
                            nc.tensor.matmul(
                                psy[dc][:],
                                lhs,
                                wd_ts[ht][:, dc * DC : (dc + 1) * DC],
                                start=(ht == 0),
                                stop=(ht == HT - 1),
                            )
                    col = t0 // P + ts_
                    for half in range(2):
                        y_t = pool.tile([P, D // 2], F32, tag="yout", bufs=2,
                                        name="y_t")
                        for j in range(2):
                            dc = half * 2 + j
                            nc.vector.tensor_scalar_mul(
                                y_t[:, j * DC : (j + 1) * DC],
                                psy[dc][:],
                                wt_t[:, col : col + 1],
                            )
                        nc.sync.dma_start(
                            out[
                                t0 + ts_ * P : t0 + (ts_ + 1) * P,
                                half * (D // 2) : (half + 1) * (D // 2),
                            ],
                            y_t[:],
                        )
                t0 += TC

    nc.finalize()
    return nc


def _get_nc(cap):
    if cap not in _built:
        _built[cap] = _build_nc(cap)
    return _built[cap]


def kernel(x, weights, Wg, Wu, Wd, indices, seq_len=None, **_unused):
    from concourse.bass_utils import run_bass_kernel_spmd

    x = np.asarray(x, dtype=np.float32)
    weights = np.asarray(weights, dtype=np.float32)
    Wg = np.asarray(Wg, dtype=np.float32)
    Wu = np.asarray(Wu, dtype=np.float32)
    Wd = np.asarray(Wd, dtype=np.float32)
    indices = np.asarray(indices)

    t, d = x.shape
    e, h, _ = Wg.shape
    k = indices.shape[1]

    # ---- host-side routing (dispatch) ----
    flat_e = indices.reshape(-1).astype(np.int64)
    flat_w = weights.reshape(-1)
    flat_t = np.repeat(np.arange(t, dtype=np.int64), k)
    order = np.argsort(flat_e, kind="stable")
    counts = np.bincount(flat_e, minlength=e)
    starts = np.zeros(e + 1, dtype=np.int64)
    starts[1:] = np.cumsum(counts)
    cap = int(-(-max(int(counts.max()), 512) // P) * P)

    tok_sorted = flat_t[order]
    w_sorted = flat_w[order]

    in_maps = []
    for ei in range(e):
        n = int(counts[ei])
        toks = tok_sorted[starts[ei] : starts[ei] + n]
        xe = np.zeros((cap, d), dtype=np.float32)
        xe[:n] = x[toks]
        wvec = np.zeros(cap, dtype=np.float32)
        wvec[:n] = w_sorted[starts[ei] : starts[ei] + n]
        # pack Wg/Wu so each h-tile block is one contiguous [128, 2048] DMA:
        # block[ht][p][k*128+hh] = Wg[e].T[k*128+p, ht*128+hh]
        WgT = Wg[ei].T  # [D, H]
        WuT = Wu[ei].T
        wg_lin = np.ascontiguousarray(
            WgT.reshape(KT, P, HT, P).transpose(2, 1, 0, 3).reshape(HT, P, KT * P)
        )
        wu_lin = np.ascontiguousarray(
            WuT.reshape(KT, P, HT, P).transpose(2, 1, 0, 3).reshape(HT, P, KT * P)
        )
        wdT = np.ascontiguousarray(Wd[ei].T)  # [H, D]
        in_maps.append(
            {
                "xT": np.ascontiguousarray(xe.T),
                "wg": wg_lin,
                "wu": wu_lin,
                "wd": wdT,
                "wt": wvec,
            }
        )

    nc = _get_nc(cap)
    trace = bool(int(os.environ.get("KERNEL_TRACE", "0")))
    res = run_bass_kernel_spmd(
        nc, in_maps, core_ids=list(range(e)), trace=trace
    )
    if trace:
        kernel.last_exec_time_ns = res.exec_time_ns
        kernel.last_results = res

    # ---- host-side combine ----
    allres = np.concatenate(
        [res.results[ei]["out"][: counts[ei]] for ei in range(e)], axis=0
    )
    inv = np.empty(t * k, dtype=np.int64)
    inv[order] = np.arange(t * k, dtype=np.int64)
    y = allres[inv].reshape(t, k, d).sum(axis=1, dtype=np.float32)
    return y



# revision 2
# speedup vs baseline: 1.1561x; 1.1561x over previous
"""MoE expert-MLP (SwiGLU) kernel for 8 Trainium2 NeuronCores.

Strategy: expert-parallel. Each of the 8 cores owns one expert's weights.
Routing slots are deduplicated on the host (a token whose K=2 picks hit the
same expert becomes ONE slot with summed weight — exactly matching the
reference's scatter-add), then dispatched per expert and padded to a fixed
capacity. Each core runs a dense [cap, D] SwiGLU MLP in bf16 (1 cycle/row
on the PE array, same rate as fp32r, half the DMA/SBUF) and scales rows by
the routing weight. The host scatter-combines the per-token contributions.

Per-core kernel: ALL weights (Wg, Wu, Wd — 17.3 MB bf16) are SBUF-resident,
DMA'd once at kernel start, so passes only stream x tiles (double-buffered)
and write output. Token passes of 512 keep every matmul's moving dim at 512
(PSUM-bank maximum), amortizing the ~13 ns per-matmul issue overhead.
  stage A: h^T[h, t] = silu(Wg @ x^T) * (Wu @ x^T)   (PSUM accum over D)
  stage B: y[t, d]  = (h^T)^T @ Wd^T, row-scaled by routing weight
"""

import sys
import os

sys.path.insert(0, "/opt/trn_rl_repo")

import numpy as np

T, D, H, E, K = 8192, 2048, 1408, 8, 2
P = 128
HT = H // P        # 11 h-tiles
KT = D // P        # 16 d-tiles
TC = 512           # tokens per pass (= PSUM bank moving-dim max)

_built = {}


def _pass_sizes(cap):
    sizes = [TC] * (cap // TC)
    r = cap % TC
    if r:
        sizes.append(r)
    return sizes


def _build_nc(cap):
    import concourse.bass as bass  # noqa: F401
    from concourse import bacc
    import concourse.mybir as mybir
    import concourse.tile as tile

    F32 = mybir.dt.float32
    BF16 = mybir.dt.bfloat16
    Silu = mybir.ActivationFunctionType.Silu
    Mult = mybir.AluOpType.mult

    sizes = _pass_sizes(cap)
    NP = len(sizes)

    nc = bacc.Bacc("TRN2", target_bir_lowering=False, debug=False)
    xT = nc.declare_dram_parameter("xT", [NP, KT, P, TC], BF16, isOutput=False)
    wg = nc.declare_dram_parameter("wg", [HT, P, KT * P], BF16, isOutput=False)
    wu = nc.declare_dram_parameter("wu", [HT, P, KT * P], BF16, isOutput=False)
    wd = nc.declare_dram_parameter("wd", [HT, P, D], BF16, isOutput=False)
    wt = nc.declare_dram_parameter("wt", [cap], F32, isOutput=False)
    out = nc.declare_dram_parameter("out", [cap, D], BF16, isOutput=True)

    with tile.TileContext(nc) as tc:
        with (
            tc.tile_pool(name="sbuf", bufs=1) as pool,
            tc.tile_pool(name="psum", bufs=1, space="PSUM") as pp,
        ):
            # ---- resident weights, loaded once ----
            # Issue order on the sync queue = service order: wg/wu h-tile 0
            # first (stage A starts on it), then the rest interleaved, then
            # Wd (only needed once stage B of pass 0 begins).
            wg_ts, wu_ts, wd_ts = [], [], []
            for ht in range(HT):
                wg_1 = pool.tile([P, KT * P], BF16, tag=f"wg{ht}", bufs=1,
                                 name=f"wg{ht}")
                nc.sync.dma_start(wg_1[:], wg[ht, :, :])
                wu_1 = pool.tile([P, KT * P], BF16, tag=f"wu{ht}", bufs=1,
                                 name=f"wu{ht}")
                nc.sync.dma_start(wu_1[:], wu[ht, :, :])
                wg_ts.append(wg_1)
                wu_ts.append(wu_1)
            for ht in range(HT):
                wd_1 = pool.tile([P, D], BF16, tag=f"wd{ht}", bufs=1,
                                 name=f"wd{ht}")
                nc.sync.dma_start(wd_1[:], wd[ht, :, :])
                wd_ts.append(wd_1)

            wt_t = pool.tile([P, cap // P], F32, tag="wt", bufs=1)
            nc.gpsimd.dma_start(wt_t[:], wt.rearrange("(n p) -> p n", p=P))

            t0 = 0
            for pi, s in enumerate(sizes):
                # x^T for this pass, one tile per d-chunk, double-buffered so
                # pass p+1's chunks prefetch during pass p's compute.
                xt_ts = []
                for dti in range(KT):
                    xt_1 = pool.tile([P, TC], BF16, tag=f"xt{dti}", bufs=2,
                                     name=f"xt{dti}")
                    nc.gpsimd.dma_start(xt_1[:], xT[pi, dti, :, :])
                    xt_ts.append(xt_1)
                h_t = pool.tile([P, HT, TC], BF16, tag="ht", bufs=2)

                # ---- stage A: h^T = silu(g^T) * u^T ----
                for ht in range(HT):
                    psg = pp.tile([P, TC], F32, tag="g", bufs=2, name="psg")
                    for dd in range(KT):
                        nc.tensor.matmul(
                            psg[:, :s],
                            wg_ts[ht][:, dd * P : (dd + 1) * P],
                            xt_ts[dd][:, :s],
                            start=(dd == 0),
                            stop=(dd == KT - 1),
                        )
                    st = pool.tile([P, TC], F32, tag="silu", bufs=2, name="st")
                    nc.scalar.activation(st[:, :s], psg[:, :s], Silu)
                    psu = pp.tile([P, TC], F32, tag="u", bufs=2, name="psu")
                    for dd in range(KT):
                        nc.tensor.matmul(
                            psu[:, :s],
                            wu_ts[ht][:, dd * P : (dd + 1) * P],
                            xt_ts[dd][:, :s],
                            start=(dd == 0),
                            stop=(dd == KT - 1),
                        )
                    nc.vector.tensor_tensor(
                        h_t[:, ht, :s], st[:, :s], psu[:, :s], op=Mult
                    )

                # ---- stage B: y = h @ Wd^T, scaled by routing weight ----
                for ts_ in range(s // P):
                    psy = [pp.tile([P, TC], F32, tag=f"y{i}", bufs=1,
                                   name=f"psy{i}") for i in range(4)]
                    for ht in range(HT):
                        lhs = h_t[:, ht, ts_ * P : (ts_ + 1) * P]
                        for dc in range(4):
                            nc.tensor.matmul(
                                psy[dc][:],
                                lhs,
                                wd_ts[ht][:, dc * TC : (dc + 1) * TC],
                                start=(ht == 0),
                                stop=(ht == HT - 1),
                            )
                    col = t0 // P + ts_
                    y_t = pool.tile([P, D], BF16, tag="yout", bufs=2)
                    for dc in range(4):
                        nc.vector.tensor_scalar_mul(
                            y_t[:, dc * TC : (dc + 1) * TC],
                            psy[dc][:],
                            wt_t[:, col : col + 1],
                        )
                    nc.sync.dma_start(
                        out[t0 + ts_ * P : t0 + (ts_ + 1) * P, :], y_t[:]
                    )
                t0 += s

    nc.finalize()
    return nc


def _get_nc(cap):
    if cap not in _built:
        _built[cap] = _build_nc(cap)
    return _built[cap]


def kernel(x, weights, Wg, Wu, Wd, indices, seq_len=None, **_unused):
    from concourse.bass_utils import run_bass_kernel_spmd
    import ml_dtypes

    BF = ml_dtypes.bfloat16

    x = np.asarray(x, dtype=np.float32)
    weights = np.asarray(weights, dtype=np.float32)
    Wg = np.asarray(Wg, dtype=np.float32)
    Wu = np.asarray(Wu, dtype=np.float32)
    Wd = np.asarray(Wd, dtype=np.float32)
    idx = np.asarray(indices).astype(np.int64)

    t, d = x.shape
    e, h, _ = Wg.shape

    # ---- host-side routing (dispatch), with same-expert slot dedup ----
    # A token routed twice to the same expert contributes once with summed
    # weight (matches the reference's scatter-add coefficient).
    e0, e1 = idx[:, 0], idx[:, 1]
    dup = e0 == e1
    tok_all = np.concatenate([np.arange(t, dtype=np.int64),
                              np.arange(t, dtype=np.int64)[~dup]])
    exp_all = np.concatenate([e0, e1[~dup]])
    w_all = np.concatenate([
        np.where(dup, weights[:, 0] + weights[:, 1], weights[:, 0]),
        weights[~dup, 1],
    ]).astype(np.float32)
    nslots = tok_all.shape[0]

    order = np.argsort(exp_all, kind="stable")
    counts = np.bincount(exp_all, minlength=e)
    starts = np.zeros(e + 1, dtype=np.int64)
    starts[1:] = np.cumsum(counts)
    cap = int(-(-max(int(counts.max()), 512) // P) * P)
    sizes = _pass_sizes(cap)
    np_ = len(sizes)

    tok_sorted = tok_all[order]
    w_sorted = w_all[order]

    xb = x.astype(BF)
    in_maps = []
    for ei in range(e):
        n = int(counts[ei])
        toks = tok_sorted[starts[ei] : starts[ei] + n]
        # pass-major x^T: xTp[pi][dti][p][tc] = x[tok(pi*TC+tc), dti*P+p]
        xe = np.zeros((np_ * TC, d), dtype=BF)
        xe[:n] = xb[toks]
        xTp = np.ascontiguousarray(
            xe.reshape(np_, TC, KT, P).transpose(0, 2, 3, 1)
        )
        wvec = np.zeros(cap, dtype=np.float32)
        wvec[:n] = w_sorted[starts[ei] : starts[ei] + n]
        # pack Wg/Wu so each h-tile block is one contiguous [128, 2048] DMA:
        # block[ht][p][k*128+hh] = Wg[e].T[k*128+p, ht*128+hh]
        WgT = Wg[ei].T  # [D, H]
        WuT = Wu[ei].T
        wg_lin = np.ascontiguousarray(
            WgT.reshape(KT, P, HT, P).transpose(2, 1, 0, 3).reshape(HT, P, KT * P),
            dtype=BF,
        )
        wu_lin = np.ascontiguousarray(
            WuT.reshape(KT, P, HT, P).transpose(2, 1, 0, 3).reshape(HT, P, KT * P),
            dtype=BF,
        )
        wd_lin = np.ascontiguousarray(
            Wd[ei].T.reshape(HT, P, D), dtype=BF
        )
        in_maps.append(
            {
                "xT": xTp,
                "wg": wg_lin,
                "wu": wu_lin,
                "wd": wd_lin,
                "wt": wvec,
            }
        )

    nc = _get_nc(cap)
    trace = bool(int(os.environ.get("KERNEL_TRACE", "0")))
    res = run_bass_kernel_spmd(
        nc, in_maps, core_ids=list(range(e)), trace=trace
    )
    if trace:
        kernel.last_exec_time_ns = res.exec_time_ns
        kernel.last_results = res

    # ---- host-side combine ----
    allres = np.concatenate(
        [res.results[ei]["out"][: counts[ei]] for ei in range(e)], axis=0
    ).astype(np.float32)
    inv = np.empty(nslots, dtype=np.int64)
    inv[order] = np.arange(nslots, dtype=np.int64)
    padded = np.concatenate([allres, np.zeros((1, d), np.float32)], axis=0)
    pos0 = inv[:t]
    pos1 = np.full(t, nslots, dtype=np.int64)
    pos1[np.arange(t)[~dup]] = inv[t:]
    y = padded[pos0] + padded[pos1]
    return y
